# revision 6
# baseline (speedup 1.0000x reference)
"""Trainium2 Bass kernel for nn_Attention_69861938037646.

Math (per batch b):
  c[b]      = b1+b2+b3 + last[b]@W2 + avg[b]@W3                  [32]
  proj[s,f] = x[b,s,:]@W1 + c[b]                                 [200,32]
  scores[s] = sigmoid(proj[s,:]) @ W4                            [200]
  out[b]    = sum_s scores[s] * x[b,s,:]                         [32]

Device layout (per core, 512 batches as 4 streams x 128 batches):
  x4[32j+e, 200g+s] = x[b0+128j+g, s, e]   (bf16, [128, 25600])
  - block-diag W1 (4x 32x32 diagonal) computes all streams' x@W1 in one
    matmul; the per-batch bias c is added into PSUM by a second matmul
    (c_gT rows as stationary, 0/1 indicator as moving operand)
  - sigmoid on ACT (one [128,1600] instruction per chunk)
  - W4 replicated block-diag matmul broadcasts scores to all e-partitions
  - DVE scalar_tensor_tensor does fused multiply+accumulate per batch
"""

import sys
from contextlib import ExitStack

import numpy as np
import ml_dtypes

sys.path.insert(0, "/opt/trn_rl_repo")

import concourse.bass as bass  # noqa: E402
import concourse.tile as tile  # noqa: E402
from concourse import bacc, mybir  # noqa: E402
from concourse.bass_utils import run_bass_kernel_spmd  # noqa: E402

B, S, E = 4096, 200, 32
NCORES = 8
BPC = B // NCORES          # 512 batches per core
NS = 4                     # partition streams
G = BPC // NS              # 128 batches per stream
COLS = G * S               # 25600 columns per core
CHUNK = 8 * S              # 1600 cols (8 batches) per psum chunk
NCHUNK = COLS // CHUNK     # 16 chunks
XCH = 2 * CHUNK            # 3200 cols per DMA tile

BF16 = mybir.dt.bfloat16
F32 = mybir.dt.float32

_CACHE = {}


def _build_program():
    nc = bacc.Bacc("TRN2", target_bir_lowering=False)
    x4 = nc.dram_tensor("x4", [128, COLS], BF16, kind="ExternalInput")
    la4 = nc.dram_tensor("la4", [128, 2 * G], BF16, kind="ExternalInput")
    w1d = nc.dram_tensor("w1d", [128, 128], BF16, kind="ExternalInput")
    w2d = nc.dram_tensor("w2d", [128, 128], BF16, kind="ExternalInput")
    w3d = nc.dram_tensor("w3d", [128, 128], BF16, kind="ExternalInput")
    w4b = nc.dram_tensor("w4b", [128, 128], BF16, kind="ExternalInput")
    b123 = nc.dram_tensor("b123", [128, 1], F32, kind="ExternalInput")
    ident = nc.dram_tensor("ident", [128, 128], F32, kind="ExternalInput")
    ind4 = nc.dram_tensor("ind4", [128, 32 * S], BF16, kind="ExternalInput")
    out4 = nc.dram_tensor("out4", [128, G], F32, kind="ExternalOutput")

    SL = [(0, 512), (512, 512), (1024, 512), (1536, 64)]

    with tile.TileContext(nc) as tc, ExitStack() as ctx:
        consts = ctx.enter_context(tc.tile_pool(name="consts", bufs=1))

        def load_const(dram, shape, dtype):
            t = consts.tile(shape, dtype, tag=dram.name)
            nc.sync.dma_start(t[:], dram[:])
            return t

        w1t = load_const(w1d, [128, 128], BF16)
        w2t = load_const(w2d, [128, 128], BF16)
        w3t = load_const(w3d, [128, 128], BF16)
        w4t = load_const(w4b, [128, 128], BF16)
        lat = load_const(la4, [128, 2 * G], BF16)
        bt = load_const(b123, [128, 1], F32)
        idt = load_const(ident, [128, 128], F32)
        indt = load_const(ind4, [128, 32 * S], BF16)

        c4 = consts.tile([128, G], F32, tag="c4")
        cgT = consts.tile([128, 128], BF16, tag="cgT")
        out_sb = consts.tile([128, G], F32, tag="out_sb")

        # ---- setup: c4[32j+f, g] = b123[f] + (last@W2 + avg@W3)[128j+g, f]
        with tc.tile_pool(name="sps", bufs=1, space="PSUM") as sps:
            cps = sps.tile([128, G], F32, tag="cps")
            nc.tensor.matmul(cps[:], w2t[:], lat[:, :G], start=True, stop=False)
            nc.tensor.matmul(cps[:], w3t[:], lat[:, G:], start=False, stop=True)
            nc.scalar.add(c4[:], cps[:], bt[:])
            # c_gT[g, 32j+f] = c4[32j+f, g]
            tpp = sps.tile([128, 128], F32, tag="tpp")
            nc.tensor.transpose(tpp[:], c4[:], idt[:])
            nc.scalar.copy(cgT[:], tpp[:])

        xpool = ctx.enter_context(tc.tile_pool(name="xpool", bufs=2))
        ppool = ctx.enter_context(tc.tile_pool(name="ppool", bufs=2, space="PSUM"))
        sgpool = ctx.enter_context(tc.tile_pool(name="sgpool", bufs=2))
        junk = ctx.enter_context(tc.tile_pool(name="junk", bufs=3))

        for q in range(NCHUNK):
            if q % 2 == 0:
                xt = xpool.tile([128, XCH], BF16, tag="xt")
                nc.sync.dma_start(xt[:], x4[:, bass.ts(q // 2, XCH)])
            xo = (q % 2) * CHUNK
            b32 = 32 * (q // 4)
            io = (q % 4) * CHUNK

            proj = ppool.tile([128, CHUNK], F32, tag="proj")
            for s0, w in SL:
                nc.tensor.matmul(
                    proj[:, s0 : s0 + w],
                    cgT[b32 : b32 + 32, :],
                    indt[b32 : b32 + 32, io + s0 : io + s0 + w],
                    start=True, stop=False,
                    tile_position=(b32, 0),
                    skip_group_check=True,
                )
            for s0, w in SL:
                nc.tensor.matmul(
                    proj[:, s0 : s0 + w],
                    w1t[:],
                    xt[:, xo + s0 : xo + s0 + w],
                    start=False, stop=True,
                    skip_group_check=True,
                )

            sig = sgpool.tile([128, CHUNK], BF16, tag="sig")
            nc.scalar.activation(
                sig[:], proj[:], mybir.ActivationFunctionType.Sigmoid
            )

            for s0, w in SL:
                nc.tensor.matmul(
                    proj[:, s0 : s0 + w],
                    w4t[:],
                    sig[:, s0 : s0 + w],
                    start=True, stop=True,
                    skip_group_check=True,
                )

            for bi in range(8):
                g = 8 * q + bi
                jt = junk.tile([128, S], BF16, tag="jt")
                nc.vector.scalar_tensor_tensor(
                    out=jt[:],
                    in0=xt[:, xo + bi * S : xo + bi * S + S],
                    scalar=0.0,
                    in1=proj[:, bi * S : bi * S + S],
                    op0=mybir.AluOpType.bypass,
                    op1=mybir.AluOpType.mult,
                    accum_out=out_sb[:, g : g + 1],
                )

        nc.sync.dma_start(out4[:], out_sb[:])

    nc.compile()
    return nc


def _prep_core(all_memory, last_memory, average_memory, i):
    b0 = i * BPC
    xs = np.ascontiguousarray(all_memory[b0 : b0 + BPC])
    x4 = (
        xs.reshape(NS, G, S, E)
        .transpose(0, 3, 1, 2)
        .reshape(128, COLS)
        .astype(ml_dtypes.bfloat16)
    )
    la = last_memory[b0 : b0 + BPC].reshape(NS, G, E).transpose(0, 2, 1).reshape(128, G)
    av = (
        average_memory[b0 : b0 + BPC]
        .reshape(NS, G, E)
        .transpose(0, 2, 1)
        .reshape(128, G)
    )
    la4 = np.concatenate([la, av], axis=1).astype(ml_dtypes.bfloat16)
    return {"x4": x4, "la4": la4}


def _shared_inputs(W1, b1, W2, b2, W3, b3, W4):
    def blockdiag(M):
        out = np.zeros((128, 128), ml_dtypes.bfloat16)
        for j in range(NS):
            out[32 * j : 32 * j + 32, 32 * j : 32 * j + 32] = M
        return out

    b123 = (np.asarray(b1) + np.asarray(b2) + np.asarray(b3)).astype(np.float32)
    ind = np.zeros((32, 32 * S), np.float32)
    for r in range(32):
        ind[r, r * S : (r + 1) * S] = 1.0
    return {
        "w1d": blockdiag(np.asarray(W1, np.float32)),
        "w2d": blockdiag(np.asarray(W2, np.float32)),
        "w3d": blockdiag(np.asarray(W3, np.float32)),
        "w4b": blockdiag(np.repeat(np.asarray(W4, np.float32).reshape(E, 1), E, 1)),
        "b123": np.tile(b123.reshape(E, 1), (NS, 1)).astype(np.float32),
        "ident": np.eye(128, dtype=np.float32),
        "ind4": np.tile(ind, (4, 1)).astype(ml_dtypes.bfloat16),
    }


def kernel(all_memory, last_memory, average_memory, mask, W1, b1, W2, b2, W3, b3, W4):
    all_memory = np.asarray(all_memory, np.float32)
    last_memory = np.asarray(last_memory, np.float32)
    average_memory = np.asarray(average_memory, np.float32)

    if "nc" not in _CACHE:
        _CACHE["nc"] = _build_program()
    nc = _CACHE["nc"]

    shared = _shared_inputs(W1, b1, W2, b2, W3, b3, W4)
    in_maps = []
    for i in range(NCORES):
        m = _prep_core(all_memory, last_memory, average_memory, i)
        m.update(shared)
        in_maps.append(m)

    res = run_bass_kernel_spmd(nc, in_maps, list(range(NCORES)))
    outs = []
    for i in range(NCORES):
        o4 = np.asarray(res.results[i]["out4"], np.float32)  # [128, G]
        outs.append(o4.reshape(NS, E, G).transpose(0, 2, 1).reshape(BPC, E))
    return np.concatenate(outs, axis=0).astype(np.float32)


# revision 14
# speedup vs baseline: 889.3280x; 889.3280x over previous
"""Trainium2 Bass kernel for nn_Attention_69861938037646.

Math (per batch b):
  c[b]      = b1+b2+b3 + last[b]@W2 + avg[b]@W3                  [32]
  proj[s,f] = x[b,s,:]@W1 + c[b]                                 [200,32]
  scores[s] = sigmoid(proj[s,:]) @ W4                            [200]
  out[b]    = sum_s scores[s] * x[b,s,:]                         [32]

Device layout (per core, 512 batches as 4 streams x 128 batches):
  x4[32j+e, 200g+s] = x[b0+128j+g, s, e]   (bf16, [128, 25600])
  - block-diag W1 (4x 32x32 diagonal) computes all streams' x@W1 in one
    matmul; the per-batch bias c is added into PSUM by a second matmul
    (c_gT rows as stationary, 0/1 indicator as moving operand)
  - sigmoid on ACT (one [128,1600] instruction per chunk)
  - W4 replicated block-diag matmul broadcasts scores to all e-partitions
  - DVE scalar_tensor_tensor does fused multiply+accumulate per batch
"""

import sys
from contextlib import ExitStack

import numpy as np
import ml_dtypes

sys.path.insert(0, "/opt/trn_rl_repo")

import concourse.bass as bass  # noqa: E402
import concourse.tile as tile  # noqa: E402
from concourse import bacc, mybir  # noqa: E402
from concourse.bass_utils import run_bass_kernel_spmd  # noqa: E402

B, S, E = 4096, 200, 32
NCORES = 8
BPC = B // NCORES          # 512 batches per core
NS = 4                     # partition streams
G = BPC // NS              # 128 batches per stream
COLS = G * S               # 25600 columns per core
CHUNK = 8 * S              # 1600 cols (8 batches) per psum chunk
NCHUNK = COLS // CHUNK     # 16 chunks

BF16 = mybir.dt.bfloat16
F32 = mybir.dt.float32

_CACHE = {}


def _build_program():
    nc = bacc.Bacc("TRN2", target_bir_lowering=False)
    # chunk-major: rows 128q..128q+128 hold chunk q, fully contiguous in DRAM
    x4 = nc.dram_tensor("x4", [128 * NCHUNK, CHUNK], BF16, kind="ExternalInput")
    la4 = nc.dram_tensor("la4", [128, 2 * G], BF16, kind="ExternalInput")
    w1d = nc.dram_tensor("w1d", [128, 128], BF16, kind="ExternalInput")
    w2d = nc.dram_tensor("w2d", [128, 128], BF16, kind="ExternalInput")
    w3d = nc.dram_tensor("w3d", [128, 128], BF16, kind="ExternalInput")
    w4b = nc.dram_tensor("w4b", [128, 128], BF16, kind="ExternalInput")
    b123 = nc.dram_tensor("b123", [128, 1], F32, kind="ExternalInput")
    ident = nc.dram_tensor("ident", [128, 128], F32, kind="ExternalInput")
    ind4 = nc.dram_tensor("ind4", [128, 32 * S], BF16, kind="ExternalInput")
    out4 = nc.dram_tensor("out4", [128, G], F32, kind="ExternalOutput")

    SL = [(0, 512), (512, 512), (1024, 512), (1536, 64)]

    with tile.TileContext(nc) as tc, ExitStack() as ctx:
        consts = ctx.enter_context(tc.tile_pool(name="consts", bufs=1))

        def load_const(dram, shape, dtype):
            t = consts.tile(shape, dtype, tag=dram.name)
            nc.sync.dma_start(t[:], dram[:])
            return t

        w1t = load_const(w1d, [128, 128], BF16)
        w2t = load_const(w2d, [128, 128], BF16)
        w3t = load_const(w3d, [128, 128], BF16)
        w4t = load_const(w4b, [128, 128], BF16)
        lat = load_const(la4, [128, 2 * G], BF16)
        bt = load_const(b123, [128, 1], F32)
        idt = load_const(ident, [128, 128], F32)
        indt = load_const(ind4, [128, 32 * S], BF16)

        c4 = consts.tile([128, G], F32, tag="c4")
        cgT = consts.tile([128, 128], BF16, tag="cgT")
        out_sb = consts.tile([128, G], F32, tag="out_sb")

        # ---- setup: c4[32j+f, g] = b123[f] + (last@W2 + avg@W3)[128j+g, f]
        with tc.tile_pool(name="sps", bufs=1, space="PSUM") as sps:
            cps = sps.tile([128, G], F32, tag="cps")
            nc.tensor.matmul(cps[:], w2t[:], lat[:, :G], start=True, stop=False)
            nc.tensor.matmul(cps[:], w3t[:], lat[:, G:], start=False, stop=True)
            nc.scalar.add(c4[:], cps[:], bt[:])
            # c_gT[g, 32j+f] = c4[32j+f, g]
            tpp = sps.tile([128, 128], F32, tag="tpp")
            nc.tensor.transpose(tpp[:], c4[:], idt[:])
            nc.scalar.copy(cgT[:], tpp[:])

        xpool = ctx.enter_context(tc.tile_pool(name="xpool", bufs=3))
        ppool = ctx.enter_context(tc.tile_pool(name="ppool", bufs=2, space="PSUM"))
        sgpool = ctx.enter_context(tc.tile_pool(name="sgpool", bufs=2))
        sxpool = ctx.enter_context(tc.tile_pool(name="sxpool", bufs=2))

        for q in range(NCHUNK):
            xt = xpool.tile([128, CHUNK], BF16, tag="xt")
            nc.sync.dma_start(xt[:], x4[bass.ts(q, 128), :])
            xo = 0
            b32 = 32 * (q // 4)
            io = (q % 4) * CHUNK

            proj = ppool.tile([128, CHUNK], F32, tag="proj")
            for s0, w in SL:
                nc.tensor.matmul(
                    proj[:, s0 : s0 + w],
                    cgT[b32 : b32 + 32, :],
                    indt[b32 : b32 + 32, io + s0 : io + s0 + w],
                    start=True, stop=False,
                    tile_position=(b32, 0),
                    skip_group_check=True,
                )
            for s0, w in SL:
                nc.tensor.matmul(
                    proj[:, s0 : s0 + w],
                    w1t[:],
                    xt[:, xo + s0 : xo + s0 + w],
                    start=False, stop=True,
                    skip_group_check=True,
                )

            sig = sgpool.tile([128, CHUNK], BF16, tag="sig")
            nc.scalar.activation(
                sig[:], proj[:], mybir.ActivationFunctionType.Sigmoid
            )

            for s0, w in SL:
                nc.tensor.matmul(
                    proj[:, s0 : s0 + w],
                    w4t[:],
                    sig[:, s0 : s0 + w],
                    start=True, stop=True,
                    skip_group_check=True,
                )

            # fused multiply then segmented per-batch sum, both on DVE
            sx = sxpool.tile([128, CHUNK], BF16, tag="sx")
            nc.vector.scalar_tensor_tensor(
                out=sx[:],
                in0=xt[:, xo : xo + CHUNK],
                scalar=0.0,
                in1=proj[:],
                op0=mybir.AluOpType.bypass,
                op1=mybir.AluOpType.mult,
            )
            nc.vector.tensor_reduce(
                out=out_sb[:, 8 * q : 8 * q + 8],
                in_=sx[:].rearrange("p (g s) -> p g s", g=8),
                axis=mybir.AxisListType.X,
                op=mybir.AluOpType.add,
            )

        nc.sync.dma_start(out4[:], out_sb[:])

    nc.compile()
    return nc


def _prep_core(all_memory, last_memory, average_memory, i):
    b0 = i * BPC
    xs = np.ascontiguousarray(all_memory[b0 : b0 + BPC])
    x4 = (
        xs.reshape(NS, G, S, E)
        .transpose(0, 3, 1, 2)
        .reshape(128, COLS)
        .astype(ml_dtypes.bfloat16)
    )
    # chunk-major DRAM layout: [16*128, 1600], rows 128q..+128 = chunk q
    x4 = np.ascontiguousarray(
        x4.reshape(128, NCHUNK, CHUNK).transpose(1, 0, 2).reshape(128 * NCHUNK, CHUNK)
    )
    la = last_memory[b0 : b0 + BPC].reshape(NS, G, E).transpose(0, 2, 1).reshape(128, G)
    av = (
        average_memory[b0 : b0 + BPC]
        .reshape(NS, G, E)
        .transpose(0, 2, 1)
        .reshape(128, G)
    )
    la4 = np.concatenate([la, av], axis=1).astype(ml_dtypes.bfloat16)
    return {"x4": x4, "la4": la4}


def _shared_inputs(W1, b1, W2, b2, W3, b3, W4):
    def blockdiag(M):
        out = np.zeros((128, 128), ml_dtypes.bfloat16)
        for j in range(NS):
            out[32 * j : 32 * j + 32, 32 * j : 32 * j + 32] = M
        return out

    b123 = (np.asarray(b1) + np.asarray(b2) + np.asarray(b3)).astype(np.float32)
    ind = np.zeros((32, 32 * S), np.float32)
    for r in range(32):
        ind[r, r * S : (r + 1) * S] = 1.0
    return {
        "w1d": blockdiag(np.asarray(W1, np.float32)),
        "w2d": blockdiag(np.asarray(W2, np.float32)),
        "w3d": blockdiag(np.asarray(W3, np.float32)),
        "w4b": blockdiag(np.repeat(np.asarray(W4, np.float32).reshape(E, 1), E, 1)),
        "b123": np.tile(b123.reshape(E, 1), (NS, 1)).astype(np.float32),
        "ident": np.eye(128, dtype=np.float32),
        "ind4": np.tile(ind, (4, 1)).astype(ml_dtypes.bfloat16),
    }


def kernel(all_memory, last_memory, average_memory, mask, W1, b1, W2, b2, W3, b3, W4):
    all_memory = np.asarray(all_memory, np.float32)
    last_memory = np.asarray(last_memory, np.float32)
    average_memory = np.asarray(average_memory, np.float32)

    if "nc" not in _CACHE:
        _CACHE["nc"] = _build_program()
    nc = _CACHE["nc"]

    shared = _shared_inputs(W1, b1, W2, b2, W3, b3, W4)
    in_maps = []
    for i in range(NCORES):
        m = _prep_core(all_memory, last_memory, average_memory, i)
        m.update(shared)
        in_maps.append(m)

    res = run_bass_kernel_spmd(nc, in_maps, list(range(NCORES)))
    outs = []
    for i in range(NCORES):
        o4 = np.asarray(res.results[i]["out4"], np.float32)  # [128, G]
        outs.append(o4.reshape(NS, E, G).transpose(0, 2, 1).reshape(BPC, E))
    return np.concatenate(outs, axis=0).astype(np.float32)


# revision 15
# speedup vs baseline: 917.6130x; 1.0318x over previous
"""Trainium2 Bass kernel for nn_Attention_69861938037646.

Math (per batch b):
  c[b]      = b1+b2+b3 + last[b]@W2 + avg[b]@W3                  [32]
  proj[s,f] = x[b,s,:]@W1 + c[b]                                 [200,32]
  scores[s] = sigmoid(proj[s,:]) @ W4                            [200]
  out[b]    = sum_s scores[s] * x[b,s,:]                         [32]

Device layout (per core, 512 batches as 4 streams x 128 batches):
  x4[32j+e, 200g+s] = x[b0+128j+g, s, e]   (bf16, [128, 25600])
  - block-diag W1 (4x 32x32 diagonal) computes all streams' x@W1 in one
    matmul; the per-batch bias c is added into PSUM by a second matmul
    (c_gT rows as stationary, 0/1 indicator as moving operand)
  - sigmoid on ACT (one [128,1600] instruction per chunk)
  - W4 replicated block-diag matmul broadcasts scores to all e-partitions
  - DVE scalar_tensor_tensor does fused multiply+accumulate per batch
"""

import sys
from contextlib import ExitStack

import numpy as np
import ml_dtypes

sys.path.insert(0, "/opt/trn_rl_repo")

import concourse.bass as bass  # noqa: E402
import concourse.tile as tile  # noqa: E402
from concourse import bacc, mybir  # noqa: E402
from concourse.bass_utils import run_bass_kernel_spmd  # noqa: E402

B, S, E = 4096, 200, 32
NCORES = 8
BPC = B // NCORES          # 512 batches per core
NS = 4                     # partition streams
G = BPC // NS              # 128 batches per stream
COLS = G * S               # 25600 columns per core
CHUNK = 8 * S              # 1600 cols (8 batches) per psum chunk
NCHUNK = COLS // CHUNK     # 16 chunks

BF16 = mybir.dt.bfloat16
F32 = mybir.dt.float32

_CACHE = {}


def _build_program():
    nc = bacc.Bacc("TRN2", target_bir_lowering=False)
    # chunk-major: rows 128q..128q+128 hold chunk q, fully contiguous in DRAM
    x4 = nc.dram_tensor("x4", [128 * NCHUNK, CHUNK], BF16, kind="ExternalInput")
    la4 = nc.dram_tensor("la4", [128, 2 * G], BF16, kind="ExternalInput")
    w1d = nc.dram_tensor("w1d", [128, 128], BF16, kind="ExternalInput")
    w2d = nc.dram_tensor("w2d", [128, 128], BF16, kind="ExternalInput")
    w3d = nc.dram_tensor("w3d", [128, 128], BF16, kind="ExternalInput")
    w4b = nc.dram_tensor("w4b", [128, 128], BF16, kind="ExternalInput")
    b123 = nc.dram_tensor("b123", [128, 1], F32, kind="ExternalInput")
    ident = nc.dram_tensor("ident", [128, 128], F32, kind="ExternalInput")
    ind4 = nc.dram_tensor("ind4", [128, 32 * S], BF16, kind="ExternalInput")
    out4 = nc.dram_tensor("out4", [128, G], F32, kind="ExternalOutput")

    SL = [(0, 512), (512, 512), (1024, 512), (1536, 64)]

    with tile.TileContext(nc) as tc, ExitStack() as ctx:
        consts = ctx.enter_context(tc.tile_pool(name="consts", bufs=1))

        def load_const(dram, shape, dtype):
            t = consts.tile(shape, dtype, tag=dram.name)
            nc.sync.dma_start(t[:], dram[:])
            return t

        w1t = load_const(w1d, [128, 128], BF16)
        w2t = load_const(w2d, [128, 128], BF16)
        w3t = load_const(w3d, [128, 128], BF16)
        w4t = load_const(w4b, [128, 128], BF16)
        lat = load_const(la4, [128, 2 * G], BF16)
        bt = load_const(b123, [128, 1], F32)
        idt = load_const(ident, [128, 128], F32)
        indt = load_const(ind4, [128, 32 * S], BF16)

        c4 = consts.tile([128, G], F32, tag="c4")
        cgT = consts.tile([128, 128], BF16, tag="cgT")
        out_sb = consts.tile([128, G], F32, tag="out_sb")

        # ---- setup: c4[32j+f, g] = b123[f] + (last@W2 + avg@W3)[128j+g, f]
        with tc.tile_pool(name="sps", bufs=1, space="PSUM") as sps:
            cps = sps.tile([128, G], F32, tag="cps")
            nc.tensor.matmul(cps[:], w2t[:], lat[:, :G], start=True, stop=False)
            nc.tensor.matmul(cps[:], w3t[:], lat[:, G:], start=False, stop=True)
            nc.scalar.add(c4[:], cps[:], bt[:])
            # c_gT[g, 32j+f] = c4[32j+f, g]
            tpp = sps.tile([128, 128], F32, tag="tpp")
            nc.tensor.transpose(tpp[:], c4[:], idt[:])
            nc.scalar.copy(cgT[:], tpp[:])

        xpool = ctx.enter_context(tc.tile_pool(name="xpool", bufs=4))
        ppool = ctx.enter_context(tc.tile_pool(name="ppool", bufs=2, space="PSUM"))
        sgpool = ctx.enter_context(tc.tile_pool(name="sgpool", bufs=3))
        sxpool = ctx.enter_context(tc.tile_pool(name="sxpool", bufs=3))

        for q in range(NCHUNK):
            xt = xpool.tile([128, CHUNK], BF16, tag="xt")
            nc.sync.dma_start(xt[:], x4[bass.ts(q, 128), :])
            xo = 0
            b32 = 32 * (q // 4)
            io = (q % 4) * CHUNK

            proj = ppool.tile([128, CHUNK], F32, tag="proj")
            for s0, w in SL:
                nc.tensor.matmul(
                    proj[:, s0 : s0 + w],
                    cgT[b32 : b32 + 32, :],
                    indt[b32 : b32 + 32, io + s0 : io + s0 + w],
                    start=True, stop=False,
                    tile_position=(b32, 0),
                    skip_group_check=True,
                )
            for s0, w in SL:
                nc.tensor.matmul(
                    proj[:, s0 : s0 + w],
                    w1t[:],
                    xt[:, xo + s0 : xo + s0 + w],
                    start=False, stop=True,
                    skip_group_check=True,
                )

            sig = sgpool.tile([128, CHUNK], BF16, tag="sig")
            nc.scalar.activation(
                sig[:], proj[:], mybir.ActivationFunctionType.Sigmoid
            )

            for s0, w in SL:
                nc.tensor.matmul(
                    proj[:, s0 : s0 + w],
                    w4t[:],
                    sig[:, s0 : s0 + w],
                    start=True, stop=True,
                    skip_group_check=True,
                )

            # fused multiply then segmented per-batch sum, both on DVE
            sx = sxpool.tile([128, CHUNK], BF16, tag="sx")
            nc.vector.scalar_tensor_tensor(
                out=sx[:],
                in0=xt[:, xo : xo + CHUNK],
                scalar=0.0,
                in1=proj[:],
                op0=mybir.AluOpType.bypass,
                op1=mybir.AluOpType.mult,
            )
            nc.vector.tensor_reduce(
                out=out_sb[:, 8 * q : 8 * q + 8],
                in_=sx[:].rearrange("p (g s) -> p g s", g=8),
                axis=mybir.AxisListType.X,
                op=mybir.AluOpType.add,
            )

        nc.sync.dma_start(out4[:], out_sb[:])

    nc.compile()
    return nc


def _prep_core(all_memory, last_memory, average_memory, i):
    b0 = i * BPC
    xs = np.ascontiguousarray(all_memory[b0 : b0 + BPC])
    x4 = (
        xs.reshape(NS, G, S, E)
        .transpose(0, 3, 1, 2)
        .reshape(128, COLS)
        .astype(ml_dtypes.bfloat16)
    )
    # chunk-major DRAM layout: [16*128, 1600], rows 128q..+128 = chunk q
    x4 = np.ascontiguousarray(
        x4.reshape(128, NCHUNK, CHUNK).transpose(1, 0, 2).reshape(128 * NCHUNK, CHUNK)
    )
    la = last_memory[b0 : b0 + BPC].reshape(NS, G, E).transpose(0, 2, 1).reshape(128, G)
    av = (
        average_memory[b0 : b0 + BPC]
        .reshape(NS, G, E)
        .transpose(0, 2, 1)
        .reshape(128, G)
    )
    la4 = np.concatenate([la, av], axis=1).astype(ml_dtypes.bfloat16)
    return {"x4": x4, "la4": la4}


def _shared_inputs(W1, b1, W2, b2, W3, b3, W4):
    def blockdiag(M):
        out = np.zeros((128, 128), ml_dtypes.bfloat16)
        for j in range(NS):
            out[32 * j : 32 * j + 32, 32 * j : 32 * j + 32] = M
        return out

    b123 = (np.asarray(b1) + np.asarray(b2) + np.asarray(b3)).astype(np.float32)
    ind = np.zeros((32, 32 * S), np.float32)
    for r in range(32):
        ind[r, r * S : (r + 1) * S] = 1.0
    return {
        "w1d": blockdiag(np.asarray(W1, np.float32)),
        "w2d": blockdiag(np.asarray(W2, np.float32)),
        "w3d": blockdiag(np.asarray(W3, np.float32)),
        "w4b": blockdiag(np.repeat(np.asarray(W4, np.float32).reshape(E, 1), E, 1)),
        "b123": np.tile(b123.reshape(E, 1), (NS, 1)).astype(np.float32),
        "ident": np.eye(128, dtype=np.float32),
        "ind4": np.tile(ind, (4, 1)).astype(ml_dtypes.bfloat16),
    }


def kernel(all_memory, last_memory, average_memory, mask, W1, b1, W2, b2, W3, b3, W4):
    all_memory = np.asarray(all_memory, np.float32)
    last_memory = np.asarray(last_memory, np.float32)
    average_memory = np.asarray(average_memory, np.float32)

    if "nc" not in _CACHE:
        _CACHE["nc"] = _build_program()
    nc = _CACHE["nc"]

    shared = _shared_inputs(W1, b1, W2, b2, W3, b3, W4)
    in_maps = []
    for i in range(NCORES):
        m = _prep_core(all_memory, last_memory, average_memory, i)
        m.update(shared)
        in_maps.append(m)

    res = run_bass_kernel_spmd(nc, in_maps, list(range(NCORES)))
    outs = []
    for i in range(NCORES):
        o4 = np.asarray(res.results[i]["out4"], np.float32)  # [128, G]
        outs.append(o4.reshape(NS, E, G).transpose(0, 2, 1).reshape(BPC, E))
    return np.concatenate(outs, axis=0).astype(np.float32)
